# revision 1
# baseline (speedup 1.0000x reference)
"""Trainium2 Bass kernel for nn_AsaTgcn (typed-GCN with concat-attention).

Math (per batch element, L=128 tokens, D=256, NT=47 dep types):
  de[i,j,:] = E'[v[i,j]]  where E' = dep_emb with row 0 zeroed, v = dep_value
  score[i,j] = (seq_i . seq_j + de[i,j] . de[j,i]) / sqrt(D)
  att = softmax(score, -1) * dep_adj
  out[i] = sum_j att[i,j] (seq_j @ W) + sum_j att[i,j] (de[j,i] @ W) + b

Key algebraic reductions (avoid the [L,L,D] de tensor entirely):
  de[i,j] . de[j,i]   = G'[v[i,j], v[j,i]],  G' = E' E'^T  (47x47 Gram table)
  sum_j att[i,j] de[j,i]@W = C @ (E'W),  C[i,t] = sum_j att[i,j]*[v[j,i]==t]

The Gram lookup is one GPSIMD ap_gather over a per-partition replicated
2209-entry flat table; C is 47 fused multiply-reduce DVE ops against a
one-hot [i,t,j] tensor built once.

Sharding: pure data parallel, batch element b -> NeuronCore b (B == 8).
"""

import os

import numpy as np

import concourse.bass as bass
import concourse.mybir as mybir
import concourse.tile as tile
from concourse import bacc
from concourse.bass_utils import run_bass_kernel_spmd
from concourse.masks import make_identity

dt = mybir.dt
Alu = mybir.AluOpType
Act = mybir.ActivationFunctionType
Axis = mybir.AxisListType

B, L, D, NT, R = 8, 128, 256, 47, 64
EPS = 1e-3
BN_SCALE = float(1.0 / np.sqrt(1.0 + EPS))
INV_SQRT_D = float(1.0 / np.sqrt(D))
KD = D // 128  # k-subtiles of the D contraction
NT2 = NT * NT  # 2209 flat Gram table size


KSTOP = int(os.environ.get("KSTOP", "99"))  # debug: stop graph after stage N


def _build_graph(nc: bass.Bass, tc: tile.TileContext):
    f32 = dt.float32
    bf16 = dt.bfloat16

    # ---------------- DRAM parameters (per-core shard) ----------------
    text_d = nc.declare_dram_parameter("text", [L, D], f32, isOutput=False)
    mask_d = nc.declare_dram_parameter("input_mask", [1, L], dt.int32, isOutput=False)
    adj_d = nc.declare_dram_parameter("dep_adj", [L, L], f32, isOutput=False)
    depv_d = nc.declare_dram_parameter("dep_value", [L, L], dt.int32, isOutput=False)
    emb_d = nc.declare_dram_parameter("dep_emb", [NT, D], f32, isOutput=False)
    gamma_d = nc.declare_dram_parameter("gamma", [1, D], f32, isOutput=False)
    beta_d = nc.declare_dram_parameter("beta", [1, D], f32, isOutput=False)
    W_d = [nc.declare_dram_parameter(f"W{i}", [D, D], f32, isOutput=False) for i in (1, 2, 3)]
    b_d = [nc.declare_dram_parameter(f"b{i}", [1, D], f32, isOutput=False) for i in (1, 2, 3)]
    fcw_d = nc.declare_dram_parameter("fc_W", [D, R], f32, isOutput=False)
    fcb_d = nc.declare_dram_parameter("fc_b", [1, R], f32, isOutput=False)
    ens_d = nc.declare_dram_parameter("ens_lin", [1, 3], f32, isOutput=False)
    out_d = nc.declare_dram_parameter("out", [1, R], f32, isOutput=True)

    # DRAM scratch for layout shuffles
    gflat_dram = nc.dram_tensor("gflat_scratch", [NT, NT], f32)

    cpool = tc.alloc_tile_pool(name="const", bufs=1)
    wpool = tc.alloc_tile_pool(name="work", bufs=3)
    pst = tc.alloc_tile_pool(name="ps_t", bufs=2, space="PSUM")
    psm = tc.alloc_tile_pool(name="ps_mm", bufs=1, space="PSUM")
    psa = tc.alloc_tile_pool(name="ps_acc", bufs=1, space="PSUM")

    def _stop(stage, src_ap):
        if KSTOP != stage:
            return False
        nc.sync.dma_start(out_d.ap(), src_ap)
        for p in (psa, psm, pst, wpool, cpool):
            p.release()
        return True

    # ---------------- constants & input loads ----------------
    ident = cpool.tile([128, 128], f32, tag="ident")
    make_identity(nc, ident[:])
    ones_row = cpool.tile([1, 128], f32, tag="ones_row")
    nc.gpsimd.memset(ones_row[:], 1.0)
    # row-0 selector: onesrow_mat.T @ x replicates x's row 0 to all partitions
    onesrow_mat = cpool.tile([128, 128], f32, tag="onesrow_mat")
    nc.gpsimd.memset(onesrow_mat[:], 0.0)
    nc.sync.dma_start(onesrow_mat[0:1, :], ones_row[:])

    iota_i = cpool.tile([128, NT], dt.int32, tag="iota_i")
    nc.gpsimd.iota(iota_i[:], pattern=[[1, NT]], base=0, channel_multiplier=0)
    iota_f = cpool.tile([128, NT], f32, tag="iota_f")
    nc.vector.tensor_copy(iota_f[:], iota_i[:])

    v_i = cpool.tile([L, L], dt.int32, tag="v_i")
    nc.sync.dma_start(v_i[:], depv_d.ap())
    emb_sb = cpool.tile([128, D], f32, tag="emb")
    nc.gpsimd.memset(emb_sb[:], 0.0)
    nc.sync.dma_start(emb_sb[0:NT, :], emb_d.ap())
    text_sb = cpool.tile([L, D], f32, tag="text")
    nc.sync.dma_start(text_sb[:], text_d.ap())
    adj_sb = cpool.tile([L, L], f32, tag="adj")
    nc.sync.dma_start(adj_sb[:], adj_d.ap())
    m_i = cpool.tile([L, 1], dt.int32, tag="m_i")
    nc.sync.dma_start(m_i[:], mask_d.ap().rearrange("o l -> l o"))
    W_sb = []
    for l in range(3):
        w = cpool.tile([128, KD, D], f32, tag=f"W{l}")
        nc.sync.dma_start(w[:], W_d[l].ap().rearrange("(ko ki) n -> ki ko n", ki=128))
        W_sb.append(w)
    fcw_sb = cpool.tile([128, KD, R], f32, tag="fcw")
    nc.sync.dma_start(fcw_sb[:], fcw_d.ap().rearrange("(ko ki) n -> ki ko n", ki=128))
    fcb_sb = cpool.tile([1, R], f32, tag="fcb")
    nc.sync.dma_start(fcb_sb[:], fcb_d.ap())
    gb_pad = cpool.tile([128, 2 * D], f32, tag="gb_pad")
    nc.gpsimd.memset(gb_pad[:], 0.0)
    nc.sync.dma_start(gb_pad[0:1, 0:D], gamma_d.ap())
    nc.sync.dma_start(gb_pad[0:1, D : 2 * D], beta_d.ap())
    ens_sb = cpool.tile([1, 3], f32, tag="ens")
    nc.sync.dma_start(ens_sb[:], ens_d.ap())

    # ---------------- one-hots / keys ----------------
    v_f = cpool.tile([L, L], f32, tag="v_f")
    nc.vector.tensor_copy(v_f[:], v_i[:])
    vT_ps = pst.tile([128, 128], f32, tag="tps")
    nc.tensor.transpose(vT_ps[:], v_f[:], ident[:])
    vT_f = cpool.tile([L, L], f32, tag="vT_f")
    nc.vector.tensor_copy(vT_f[:], vT_ps[:])

    # key[i,j] = v[i,j]*47 + v[j,i] -> int16. Used directly as ap_gather idxs:
    # the 16-partition wrap makes core c's m-th gather read key[16c + m%16, m//16],
    # so gather output column 16j+k holds score2[16c+k, j].
    key_f = wpool.tile([L, L], f32, tag="key_f")
    nc.vector.scalar_tensor_tensor(key_f[:], v_f[:], float(NT), vT_f[:], Alu.mult, Alu.add)
    idx_sb = cpool.tile([L, L], dt.int16, tag="idx")
    nc.vector.tensor_copy(idx_sb[:], key_f[:])

    if _stop(1, v_f[0:1, 0:R]):
        return

    # ---------------- Gram table & per-layer embedding projections ----------------
    # E'^T [128, KD, 47] with type-0 column zeroed
    et_sb = cpool.tile([128, KD, NT], f32, tag="et")
    for k in range(KD):
        tp = pst.tile([128, 128], f32, tag="tps")
        nc.tensor.transpose(tp[:], emb_sb[:, k * 128 : (k + 1) * 128], ident[:])
        nc.vector.tensor_copy(et_sb[:, k, :], tp[:, 0:NT])
    nc.gpsimd.memset(et_sb[:, :, 0:1], 0.0)

    g_ps = psm.tile([NT, NT], f32, tag="mm_small")
    for k in range(KD):
        nc.tensor.matmul(g_ps[:], et_sb[:, k, :], et_sb[:, k, :], start=(k == 0), stop=(k == KD - 1))
    g_sb = cpool.tile([NT, NT], f32, tag="g_sb")
    nc.scalar.mul(g_sb[:], g_ps[:], INV_SQRT_D)  # fold 1/sqrt(D) into the table
    nc.sync.dma_start(gflat_dram.ap(), g_sb[:])

    # replicate the flat table to all 128 partitions: one DMA re-reading the
    # same 8.8KB DRAM row per partition (step-0 outer dim on the source)
    gtab = cpool.tile([128, NT2], f32, tag="gtab")
    nc.sync.dma_start(
        gtab[:], bass.AP(gflat_dram, 0, [[0, 128], [1, NT2]])
    )

    # EW[l] rows 0:47 = E' @ W_l (row 0 = 0), row 47 = bias b_l, rows 48+ zero
    ew_sb = []
    for l in range(3):
        ew = cpool.tile([128, D], f32, tag=f"ew{l}", name=f"ew{l}")
        nc.gpsimd.memset(ew[:], 0.0)
        ewp = psm.tile([NT, D], f32, tag="mm_wide")
        for k in range(KD):
            nc.tensor.matmul(
                ewp[:], et_sb[:, k, :], W_sb[l][:, k, :],
                start=(k == 0), stop=(k == KD - 1),
            )
        nc.vector.tensor_copy(ew[0:NT, :], ewp[:])
        nc.sync.dma_start(ew[NT : NT + 1, :], b_d[l].ap())
        ew_sb.append(ew)

    # C^T tile: row 47 fixed at 1.0 (bias row), rows 0:47 written per layer,
    # rows 48+ zero so the K=128 matmul contraction is unaffected
    ct_sb = cpool.tile([128, 128], f32, tag="ct")
    nc.gpsimd.memset(ct_sb[:], 0.0)
    nc.sync.dma_start(ct_sb[NT : NT + 1, :], ones_row[:])

    if _stop(2, gtab[0:1, 0:R]):
        return

    # ---------------- score2 via gathered Gram table ----------------
    gath = cpool.tile([128, 16 * L], f32, tag="gath")
    nc.gpsimd.ap_gather(
        gath[:], gtab[:], idx_sb[:], channels=128, num_elems=NT2, d=1, num_idxs=16 * L
    )
    # gath[16c+p, 16j+k] == score2[16c+k, j] for every p in the core group.
    # Extract with 16 row-masked accumulations: rowmask_k[p] = (p % 16 == k).
    pm_i = cpool.tile([128, 16], dt.int32, tag="pm_i")
    nc.gpsimd.iota(pm_i[:], pattern=[[0, 16]], base=0, channel_multiplier=1)
    pm16_i = cpool.tile([128, 16], dt.int32, tag="pm16_i")
    nc.vector.tensor_scalar(pm16_i[:], pm_i[:], 15, None, Alu.bitwise_and)
    km = cpool.tile([128, 16], f32, tag="km")
    nc.vector.tensor_tensor(km[:], pm16_i[:], iota_i[:, 0:16], Alu.is_equal)

    gath3 = gath[:].rearrange("p (j k) -> p k j", k=16)
    s2_a = cpool.tile([L, L], f32, tag="s2_a")
    s2_b = cpool.tile([L, L], f32, tag="s2_b")
    last_ext = nc.vector.tensor_scalar(s2_a[:], gath3[:, 0, :], km[:, 0:1], None, Alu.mult)
    cur, nxt = s2_a, s2_b
    for k in range(1, 16):
        last_ext = nc.vector.scalar_tensor_tensor(
            nxt[:], gath3[:, k, :], km[:, k : k + 1], cur[:], Alu.mult, Alu.add
        )
        cur, nxt = nxt, cur
    s2_sb = cur

    # S_T3[i, t, j] = (v[j,i] == t), one DVE compare op (bf16: exact 0/1).
    # Deliberately after the score2 chain: layer 0 needs s2 before st3.
    vT_bf = cpool.tile([L, L], bf16, tag="vT_bf")
    nc.vector.tensor_copy(vT_bf[:], vT_f[:])
    iota_bf = cpool.tile([128, NT], bf16, tag="iota_bf")
    nc.vector.tensor_copy(iota_bf[:], iota_f[:])
    st3 = cpool.tile([L, NT, L], bf16, tag="st3")
    nc.vector.tensor_tensor(
        st3[:],
        vT_bf[:, None, :].to_broadcast((L, NT, L)),
        iota_bf[:, :, None].to_broadcast((L, NT, L)),
        Alu.is_equal,
    )

    if _stop(3, s2_sb[0:1, 0:R]):
        return

    # ---------------- mask / ensemble weights ----------------
    m_f = cpool.tile([L, 1], f32, tag="m_f")
    nc.vector.tensor_copy(m_f[:], m_i[:])
    cnt_ps = psm.tile([1, 1], f32, tag="mm_small")
    nc.tensor.matmul(cnt_ps[:], m_f[:], m_f[:])
    rcnt = cpool.tile([1, 1], f32, tag="rcnt")
    nc.vector.tensor_scalar_add(rcnt[:], cnt_ps[:], 1e-10)
    nc.vector.reciprocal(rcnt[:], rcnt[:])

    nmx3 = wpool.tile([1, 1], f32, tag="nmx3")
    nc.vector.tensor_reduce(nmx3[:], ens_sb[:], axis=Axis.X, op=Alu.max, negate=True)
    e3 = wpool.tile([1, 3], f32, tag="e3")
    z3 = wpool.tile([1, 1], f32, tag="z3")
    nc.scalar.activation(e3[:], ens_sb[:], Act.Exp, bias=nmx3[:], scale=1.0, accum_out=z3[:])
    rz3 = wpool.tile([1, 1], f32, tag="rz3")
    nc.vector.reciprocal(rz3[:], z3[:])
    wc_pad = cpool.tile([128, 3], f32, tag="wc_pad")
    nc.gpsimd.memset(wc_pad[:], 0.0)
    nc.vector.tensor_scalar(wc_pad[0:1, :], e3[:], rz3[:], rcnt[:], Alu.mult, Alu.mult)
    wbc_ps = psm.tile([128, 3], f32, tag="mm_small")
    nc.tensor.matmul(wbc_ps[:], onesrow_mat[:], wc_pad[:])
    wbc = cpool.tile([128, 3], f32, tag="wbc")
    nc.vector.tensor_copy(wbc[:], wbc_ps[:])
    m_w = cpool.tile([L, 3], f32, tag="m_w")
    nc.vector.tensor_tensor(m_w[:], m_f[:].to_broadcast((L, 3)), wbc[:], Alu.mult)

    # ---------------- batch norm (inference) ----------------
    gbbc_ps = psm.tile([128, 2 * D], f32, tag="mm_bcast")
    nc.tensor.matmul(gbbc_ps[:], onesrow_mat[:], gb_pad[:])
    gbbc = cpool.tile([128, 2 * D], f32, tag="gbbc")
    nc.vector.tensor_copy(gbbc[:], gbbc_ps[:])

    seq = cpool.tile([L, D], f32, tag="seq0")
    nc.vector.tensor_tensor(seq[:], text_sb[:], gbbc[:, 0:D], Alu.mult)
    nc.vector.scalar_tensor_tensor(seq[:], seq[:], BN_SCALE, gbbc[:, D : 2 * D], Alu.mult, Alu.add)

    seqT = cpool.tile([128, KD, 128], f32, tag="seqT0")
    for k in range(KD):
        tp = pst.tile([128, 128], f32, tag="tps")
        nc.tensor.transpose(tp[:], seq[:, k * 128 : (k + 1) * 128], ident[:])
        nc.scalar.copy(seqT[:, k, :], tp[:])

    ens_ps = [psa.tile([128, 1], f32, tag=f"ensT{k}", name=f"ensT{k}") for k in range(KD)]

    if _stop(4, seq[0:1, 0:R]):
        return

    # ---------------- the three TGCN layers ----------------
    for l in range(3):
        s1_ps = psm.tile([L, L], f32, tag="mm_out")
        for k in range(KD):
            nc.tensor.matmul(s1_ps[:], seqT[:, k, :], seqT[:, k, :], start=(k == 0), stop=(k == KD - 1))
        score = wpool.tile([L, L], f32, tag="score")
        nc.vector.scalar_tensor_tensor(score[:], s1_ps[:], INV_SQRT_D, s2_sb[:], Alu.mult, Alu.add)

        nmx = wpool.tile([L, 1], f32, tag="nmx")
        nc.vector.tensor_reduce(nmx[:], score[:], axis=Axis.X, op=Alu.max, negate=True)
        e_sb = wpool.tile([L, L], f32, tag="e_sb")
        z = wpool.tile([L, 1], f32, tag="z")
        nc.scalar.activation(e_sb[:], score[:], Act.Exp, bias=nmx[:], scale=1.0, accum_out=z[:])
        rz = wpool.tile([L, 1], f32, tag="rz")
        nc.vector.reciprocal(rz[:], z[:])
        att = wpool.tile([L, L], f32, tag="att")
        nc.vector.scalar_tensor_tensor(att[:], e_sb[:], rz[:], adj_sb[:], Alu.mult, Alu.mult)

        atT_ps = pst.tile([128, 128], f32, tag="tps")
        nc.tensor.transpose(atT_ps[:], att[:], ident[:])
        attT = wpool.tile([L, L], f32, tag="attT")
        nc.vector.tensor_copy(attT[:], atT_ps[:])

        tw_ps = psm.tile([L, D], f32, tag="mm_wide")
        for k in range(KD):
            nc.tensor.matmul(
                tw_ps[:], seqT[:, k, :], W_sb[l][:, k, :],
                start=(k == 0), stop=(k == KD - 1),
            )
        tw = wpool.tile([L, D], f32, tag="tw")
        nc.scalar.copy(tw[:], tw_ps[:])

        # C[i,t] = sum_j att[i,j] * [v[j,i]==t]   (bf16 elementwise product)
        att_bf = wpool.tile([L, L], bf16, tag="att_bf")
        nc.vector.tensor_copy(att_bf[:], att[:])
        prod = cpool.tile([L, NT, L], bf16, tag="prod", name="prod")
        nc.vector.tensor_tensor(
            prod[:], att_bf[:, None, :].to_broadcast((L, NT, L)), st3[:], Alu.mult
        )
        # binary halving tree over j in bf16 (2x DVE), final 8-wide reduce
        c_sb = wpool.tile([L, NT], f32, tag="c_sb")
        w = L
        while w > 8:
            h = w // 2
            nc.vector.tensor_tensor(
                prod[:, :, 0:h], prod[:, :, 0:h], prod[:, :, h:w], Alu.add
            )
            w = h
        nc.vector.tensor_reduce(
            c_sb[:], prod[:, :, 0:8], axis=Axis.X, op=Alu.add
        )
        ct_ps = pst.tile([128, 128], f32, tag="tps")
        nc.tensor.transpose(ct_ps[0:NT, :], c_sb[:], ident[:])
        nc.vector.tensor_copy(ct_sb[0:NT, :], ct_ps[0:NT, :])

        out_ps = psm.tile([L, D], f32, tag="mm_out")
        nc.tensor.matmul(out_ps[:], attT[:], tw[:], start=True, stop=False)
        nc.tensor.matmul(out_ps[:], ct_sb[:], ew_sb[l][:], start=False, stop=True)

        seq_n = wpool.tile([L, D], f32, tag="seq_n")
        nc.scalar.activation(seq_n[:], out_ps[:], Act.Relu)

        # masked-mean pool, softmax(ens)-weighted, accumulated in PSUM over layers
        for k in range(KD):
            nc.tensor.matmul(
                ens_ps[k][:], seq_n[:, k * 128 : (k + 1) * 128], m_w[:, l : l + 1],
                start=(l == 0), stop=(l == 2),
            )

        if l < 2:
            seqT = wpool.tile([128, KD, 128], f32, tag="seqT_n")
            for k in range(KD):
                tp = pst.tile([128, 128], f32, tag="tps")
                nc.tensor.transpose(tp[:], seq_n[:, k * 128 : (k + 1) * 128], ident[:])
                nc.scalar.copy(seqT[:, k, :], tp[:])
        seq = seq_n
        if l == 0 and _stop(5, seq[0:1, 0:R]):
            return

    # ---------------- final fc ----------------
    ensT = wpool.tile([128, KD, 1], f32, tag="ensT_sb")
    for k in range(KD):
        nc.vector.tensor_copy(ensT[:, k, :], ens_ps[k][:])
    fin_ps = psm.tile([1, R], f32, tag="mm_small")
    for k in range(KD):
        nc.tensor.matmul(fin_ps[:], ensT[:, k, :], fcw_sb[:, k, :], start=(k == 0), stop=(k == KD - 1))
    out_sb = wpool.tile([1, R], f32, tag="out_sb")
    nc.vector.tensor_tensor(out_sb[:], fin_ps[:], fcb_sb[:], Alu.add)
    nc.sync.dma_start(out_d.ap(), out_sb[:])

    for p in (psa, psm, pst, wpool, cpool):
        p.release()


_NC_CACHE = {}


def build_nc():
    if "nc" not in _NC_CACHE:
        nc = bacc.Bacc("TRN2", target_bir_lowering=False, debug=False)
        with tile.TileContext(nc) as tc:
            _build_graph(nc, tc)
        nc.compile()
        _NC_CACHE["nc"] = nc
    return _NC_CACHE["nc"]


def _in_maps(inputs):
    maps = []
    for c in range(B):
        m = {
            "text": np.ascontiguousarray(inputs["text"][c], np.float32),
            "input_mask": np.ascontiguousarray(inputs["input_mask"][c : c + 1], np.int32),
            "dep_adj": np.ascontiguousarray(inputs["dep_adj"][c], np.float32),
            "dep_value": np.ascontiguousarray(inputs["dep_value"][c], np.int32),
            "dep_emb": np.ascontiguousarray(inputs["dep_emb"], np.float32),
            "gamma": np.ascontiguousarray(inputs["gamma"][None, :], np.float32),
            "beta": np.ascontiguousarray(inputs["beta"][None, :], np.float32),
            "W1": np.ascontiguousarray(inputs["W1"], np.float32),
            "b1": np.ascontiguousarray(inputs["b1"][None, :], np.float32),
            "W2": np.ascontiguousarray(inputs["W2"], np.float32),
            "b2": np.ascontiguousarray(inputs["b2"][None, :], np.float32),
            "W3": np.ascontiguousarray(inputs["W3"], np.float32),
            "b3": np.ascontiguousarray(inputs["b3"][None, :], np.float32),
            "fc_W": np.ascontiguousarray(inputs["fc_W"], np.float32),
            "fc_b": np.ascontiguousarray(inputs["fc_b"][None, :], np.float32),
            "ens_lin": np.ascontiguousarray(inputs["ens_lin"][None, :], np.float32),
        }
        maps.append(m)
    return maps


def kernel(**inputs):
    nc = build_nc()
    res = run_bass_kernel_spmd(nc, _in_maps(inputs), core_ids=list(range(B)))
    return np.concatenate([r["out"] for r in res.results], axis=0)


def kernel_traced(**inputs):
    """Same as kernel() but returns (output, exec_time_ns)."""
    nc = build_nc()
    res = run_bass_kernel_spmd(
        nc, _in_maps(inputs), core_ids=list(range(B)), trace=True
    )
    out = np.concatenate([r["out"] for r in res.results], axis=0)
    return out, res.exec_time_ns



# revision 30
# speedup vs baseline: 1.0543x; 1.0543x over previous
"""Trainium2 Bass kernel for nn_AsaTgcn (typed-GCN with concat-attention).

Math (per batch element, L=128 tokens, D=256, NT=47 dep types):
  de[i,j,:] = E'[v[i,j]]  where E' = dep_emb with row 0 zeroed, v = dep_value
  score[i,j] = (seq_i . seq_j + de[i,j] . de[j,i]) / sqrt(D)
  att = softmax(score, -1) * dep_adj
  out[i] = sum_j att[i,j] (seq_j @ W) + sum_j att[i,j] (de[j,i] @ W) + b

Key algebraic reductions (avoid the [L,L,D] de tensor entirely):
  de[i,j] . de[j,i]   = G'[v[i,j], v[j,i]],  G' = E' E'^T  (47x47 Gram table)
  sum_j att[i,j] de[j,i]@W = C @ (E'W),  C[i,t] = sum_j att[i,j]*[v[j,i]==t]

v2 structure (vs v1):
  - score2 extraction: bf16 Gram table, gather, then mask + binary tree over
    the 16-wrap axis (replaces the 16-op serial masked-accumulate chain).
  - one-hot S ([i, chunk, t, j32] layout): 2 chunks built by DVE is_equal,
    2 by GPSIMD local_scatter (off the DVE critical path).
  - per-layer C: 4 chunked bf16 broadcast-multiplies (2x DVE mode) + add tree.
  - fp32r (full-precision fp32, 1 cyc/row at >=256 moving cols) for all wide
    matmuls off the score-critical path: tw, out1/out2, E'W.
  - partition-broadcast APs for BN gamma/beta and pool-weight rows (kills the
    ones-matmul broadcast trick and its PSUM evacuations).
  - bias folded as ct row 47 = 1.0 with K=48 contraction (no zero padding).
  - dual DMA queues: gram-table roundtrip on the Act HWDGE queue, bulk input
    loads on the SP queue.

Sharding: pure data parallel, batch element b -> NeuronCore b (B == 8).
"""

import os

import numpy as np

import concourse.bass as bass
import concourse.mybir as mybir
import concourse.tile as tile
from concourse import bacc
from concourse.bass_utils import run_bass_kernel_spmd
from concourse.masks import make_identity

dt = mybir.dt
Alu = mybir.AluOpType
Act = mybir.ActivationFunctionType
Axis = mybir.AxisListType

B, L, D, NT, R = 8, 128, 256, 47, 64
EPS = 1e-3
BN_SCALE = float(1.0 / np.sqrt(1.0 + EPS))
INV_SQRT_D = float(1.0 / np.sqrt(D))
KD = D // 128  # k-subtiles of the D contraction
NT2 = NT * NT  # 2209 flat Gram table size
JC = 32        # j-chunk width of the one-hot layout
NCH = L // JC  # 4 chunks

KSTOP = int(os.environ.get("KSTOP", "99"))  # debug: stop graph after stage N


def _build_graph(nc: bass.Bass, tc: tile.TileContext):
    f32 = dt.float32
    f32r = dt.float32r
    bf16 = dt.bfloat16

    # ---------------- DRAM parameters (per-core shard) ----------------
    text_d = nc.declare_dram_parameter("text", [L, D], f32, isOutput=False)
    mask_d = nc.declare_dram_parameter("input_mask", [1, L], dt.int32, isOutput=False)
    adj_d = nc.declare_dram_parameter("dep_adj", [L, L], f32, isOutput=False)
    depv_d = nc.declare_dram_parameter("dep_value", [L, L], dt.int32, isOutput=False)
    emb_d = nc.declare_dram_parameter("dep_emb", [NT, D], f32, isOutput=False)
    gamma_d = nc.declare_dram_parameter("gamma", [1, D], f32, isOutput=False)
    beta_d = nc.declare_dram_parameter("beta", [1, D], f32, isOutput=False)
    W_d = [nc.declare_dram_parameter(f"W{i}", [D, D], f32r, isOutput=False) for i in (1, 2, 3)]
    b_d = [nc.declare_dram_parameter(f"b{i}", [1, D], f32r, isOutput=False) for i in (1, 2, 3)]
    fcw_d = nc.declare_dram_parameter("fc_W", [D, R], f32, isOutput=False)
    fcb_d = nc.declare_dram_parameter("fc_b", [1, R], f32, isOutput=False)
    ens_d = nc.declare_dram_parameter("ens_lin", [1, 3], f32, isOutput=False)
    out_d = nc.declare_dram_parameter("out", [1, R], f32, isOutput=True)

    gflat_dram = nc.dram_tensor("gflat_scratch", [NT, NT], f32)

    cpool = tc.alloc_tile_pool(name="const", bufs=1)
    wpool = tc.alloc_tile_pool(name="work", bufs=3)
    pst = tc.alloc_tile_pool(name="ps_t", bufs=2, space="PSUM")
    psm = tc.alloc_tile_pool(name="ps_mm", bufs=1, space="PSUM")
    psa = tc.alloc_tile_pool(name="ps_acc", bufs=1, space="PSUM")

    def _stop(stage, src_ap):
        if KSTOP != stage:
            return False
        nc.sync.dma_start(out_d.ap(), src_ap)
        for p in (psa, psm, pst, wpool, cpool):
            p.release()
        return True

    # ---------------- input DMAs (SP queue, need-ordered) ----------------
    v_i = cpool.tile([L, L], dt.int32, tag="v_i")
    nc.sync.dma_start(v_i[:], depv_d.ap())
    # E' = dep_emb with row 0 zeroed: skip row 0 in the load, memset covers it
    emb_sb = cpool.tile([128, D], f32, tag="emb")
    nc.gpsimd.memset(emb_sb[:], 0.0)
    nc.sync.dma_start(emb_sb[1:NT, :], emb_d.ap()[1:NT, :])
    text_sb = cpool.tile([L, D], f32, tag="text")
    nc.sync.dma_start(text_sb[:], text_d.ap())
    adj_sb = cpool.tile([L, L], f32, tag="adj")
    nc.sync.dma_start(adj_sb[:], adj_d.ap())
    m_i = cpool.tile([L, 1], dt.int32, tag="m_i")
    nc.sync.dma_start(m_i[:], mask_d.ap().rearrange("o l -> l o"))
    gb_row = cpool.tile([1, 2 * D], f32, tag="gb_row")
    nc.sync.dma_start(gb_row[0:1, 0:D], gamma_d.ap())
    nc.sync.dma_start(gb_row[0:1, D : 2 * D], beta_d.ap())
    ens_sb = cpool.tile([1, 3], f32, tag="ens")
    nc.sync.dma_start(ens_sb[:], ens_d.ap())

    # ---------------- constants ----------------
    ident = cpool.tile([128, 128], f32, tag="ident")
    make_identity(nc, ident[:])
    ident_bf = cpool.tile([128, 128], bf16, tag="ident_bf")
    nc.vector.tensor_copy(ident_bf[:], ident[:])

    iota_i = cpool.tile([128, NT], dt.int32, tag="iota_i")
    nc.gpsimd.iota(iota_i[:], pattern=[[1, NT]], base=0, channel_multiplier=0)
    iota_bf = cpool.tile([128, NT], bf16, tag="iota_bf")
    nc.vector.tensor_copy(iota_bf[:], iota_i[:])

    # km[p, k] = (p % 16 == k), bf16 — the 16-wrap selector
    pm_i = cpool.tile([128, 16], dt.int32, tag="pm_i")
    nc.gpsimd.iota(pm_i[:], pattern=[[0, 16]], base=0, channel_multiplier=1)
    pm16_i = cpool.tile([128, 16], dt.int32, tag="pm16_i")
    nc.vector.tensor_scalar(pm16_i[:], pm_i[:], 15, None, Alu.bitwise_and)
    km_bf = cpool.tile([128, 16], bf16, tag="km_bf")
    nc.vector.tensor_tensor(km_bf[:], pm16_i[:], iota_i[:, 0:16], Alu.is_equal)

    # jmod[p, j] = j % 32 (for the local_scatter index build)
    jmod_i = cpool.tile([128, L], dt.int32, tag="jmod_i")
    nc.gpsimd.iota(jmod_i[:], pattern=[[0, NCH], [1, JC]], base=0, channel_multiplier=0)

    ones_bf = cpool.tile([128, JC], bf16, tag="ones_bf")
    nc.gpsimd.memset(ones_bf[:], 1.0)

    # ---------------- early DVE: keys / vT / bn ----------------
    v_f = cpool.tile([L, L], f32, tag="v_f")
    nc.vector.tensor_copy(v_f[:], v_i[:])
    vT_ps = pst.tile([128, 128], f32, tag="tps")
    nc.tensor.transpose(vT_ps[:], v_f[:], ident[:])
    vT_f = cpool.tile([L, L], f32, tag="vT_f")
    nc.vector.tensor_copy(vT_f[:], vT_ps[:])
    vT_bf = cpool.tile([L, L], bf16, tag="vT_bf")
    nc.vector.tensor_copy(vT_bf[:], vT_f[:])

    # gather key: key[i,j] = v[i,j]*47 + v[j,i]
    key_f = wpool.tile([L, L], f32, tag="key_f")
    nc.vector.scalar_tensor_tensor(key_f[:], v_f[:], float(NT), vT_f[:], Alu.mult, Alu.add)
    idx_sb = cpool.tile([L, L], dt.int16, tag="idx")
    nc.vector.tensor_copy(idx_sb[:], key_f[:])

    # local_scatter idx: idx_sc[i, j] = vT[i,j]*32 + (j % 32)
    jmod_f = cpool.tile([128, L], f32, tag="jmod_f")
    nc.vector.tensor_copy(jmod_f[:], jmod_i[:])
    idxsc_f = wpool.tile([L, L], f32, tag="idxsc_f")
    nc.vector.scalar_tensor_tensor(idxsc_f[:], vT_f[:], float(JC), jmod_f[:], Alu.mult, Alu.add)
    idx_sc = cpool.tile([L, L], dt.int16, tag="idx_sc")
    nc.vector.tensor_copy(idx_sc[:], idxsc_f[:])

    # batch norm (inference): seq = gamma*text*BN_SCALE + beta. Broadcast the
    # [1, 2D] gamma/beta row to all partitions via a K=1 outer product.
    ones_col = cpool.tile([1, 128], f32, tag="ones_col")
    nc.gpsimd.memset(ones_col[:], 1.0)
    gbbc_ps = psm.tile([128, 2 * D], f32, tag="mm_wide")
    nc.tensor.matmul(gbbc_ps[:], ones_col[:], gb_row[:])
    gbbc = cpool.tile([128, 2 * D], f32, tag="gbbc")
    nc.scalar.copy(gbbc[:], gbbc_ps[:])
    seq = cpool.tile([L, D], f32, tag="seq0")
    nc.vector.tensor_tensor(seq[:], text_sb[:], gbbc[:, 0:D], Alu.mult)
    nc.vector.scalar_tensor_tensor(seq[:], seq[:], BN_SCALE, gbbc[:, D : 2 * D], Alu.mult, Alu.add)

    seqT = cpool.tile([128, KD, 128], f32, tag="seqT0")
    seqT_r = cpool.tile([128, KD, 128], f32r, tag="seqT0r")
    for k in range(KD):
        tp = pst.tile([128, 128], f32, tag="tps")
        nc.tensor.transpose(tp[:], seq[:, k * 128 : (k + 1) * 128], ident[:])
        nc.scalar.copy(seqT[:, k, :], tp[:])
        nc.scalar.copy(seqT_r[:, k, :], tp[:])

    if _stop(1, seq[0:1, 0:R]):
        return

    # ---------------- one-hot S: st3[i, c, t, j32] = [vT[i, 32c+j]==t] ----
    st3 = cpool.tile([L, NCH, NT, JC], bf16, tag="st3")
    # chunks 0,1 on DVE (fills the wait-for-gram-table window)
    for c in range(2):
        nc.vector.tensor_tensor(
            st3[:, c, :, :],
            vT_bf[:, None, JC * c : JC * (c + 1)].to_broadcast((L, NT, JC)),
            iota_bf[:, :, None].to_broadcast((L, NT, JC)),
            Alu.is_equal,
        )

    # ---------------- Gram table (PE) + replicated bf16 load ----------------
    et_sb = cpool.tile([128, KD, NT], f32r, tag="et")
    et_f = cpool.tile([128, KD, NT], f32, tag="et_f")
    for k in range(KD):
        tp = pst.tile([128, 128], f32, tag="tps")
        nc.tensor.transpose(tp[:], emb_sb[:, k * 128 : (k + 1) * 128], ident[:])
        nc.scalar.copy(et_sb[:, k, :], tp[:, 0:NT])
        nc.scalar.copy(et_f[:, k, :], tp[:, 0:NT])

    g_ps = psm.tile([NT, NT], f32, tag="mm_small")
    for k in range(KD):
        nc.tensor.matmul(g_ps[:], et_f[:, k, :], et_f[:, k, :], start=(k == 0), stop=(k == KD - 1))
    g_sb = cpool.tile([NT, NT], f32, tag="g_sb")
    nc.scalar.mul(g_sb[:], g_ps[:], INV_SQRT_D)  # fold 1/sqrt(D)

    # gram roundtrip on the Act HWDGE queue (SP queue carries the bulk loads)
    nc.scalar.dma_start(gflat_dram.ap(), g_sb[:])
    gtab = cpool.tile([128, NT2], f32, tag="gtab")
    nc.scalar.dma_start(gtab[:], bass.AP(gflat_dram, 0, [[0, 128], [1, NT2]]))

    if _stop(2, seq[0:1, 0:R]):
        return

    # ---------------- score2 via gathered Gram table ----------------
    # gath[16g+p, 16j+k] == score2[16g+k, j] for every p in core group g
    gath = cpool.tile([128, 16 * L], f32, tag="gath")
    nc.gpsimd.ap_gather(
        gath[:], gtab[:], idx_sb[:], channels=128, num_elems=NT2, d=1, num_idxs=16 * L
    )

    # one-hot chunks 2,3 on GPSIMD (after the gather on the Pool queue)
    for c in range(2, NCH):
        nc.gpsimd.local_scatter(
            st3[:, c, :, :],
            ones_bf[:],
            idx_sc[:, JC * c : JC * (c + 1)],
            channels=128,
            num_elems=NT * JC,
            num_idxs=JC,
        )

    # extract s2[p, j] = gath[p, 16j + p%16]: mask by km (bf16 out) then a
    # bf16 2x-mode tree over the wrap axis k
    mk = cpool.tile([L, L, 16], bf16, tag="mk")
    nc.vector.tensor_tensor(
        mk[:],
        gath[:].rearrange("p (j k) -> p j k", k=16),
        km_bf[:, None, :].to_broadcast((L, L, 16)),
        Alu.mult,
    )
    nc.vector.tensor_tensor(mk[:, :, 0:8], mk[:, :, 0:8], mk[:, :, 8:16], Alu.add)
    nc.vector.tensor_tensor(mk[:, :, 0:4], mk[:, :, 0:4], mk[:, :, 4:8], Alu.add)
    nc.vector.tensor_tensor(mk[:, :, 0:2], mk[:, :, 0:2], mk[:, :, 2:4], Alu.add)
    s2_sb = cpool.tile([L, L], f32, tag="s2_sb")
    nc.vector.tensor_tensor(s2_sb[:], mk[:, :, 0], mk[:, :, 1], Alu.add)

    if _stop(3, s2_sb[0:1, 0:R]):
        return

    # ---------------- weights / EW / mask / ens ----------------
    W_sb = []
    for l in range(3):
        w = cpool.tile([128, KD, D], f32r, tag=f"W{l}")
        nc.sync.dma_start(w[:], W_d[l].ap().rearrange("(ko ki) n -> ki ko n", ki=128))
        W_sb.append(w)
    fcw_sb = cpool.tile([128, KD, R], f32, tag="fcw")
    nc.sync.dma_start(fcw_sb[:], fcw_d.ap().rearrange("(ko ki) n -> ki ko n", ki=128))
    fcb_sb = cpool.tile([1, R], f32, tag="fcb")
    nc.sync.dma_start(fcb_sb[:], fcb_d.ap())

    # EW[l]: rows 0:47 = E' @ W_l (fp32r), row 47 = bias b_l
    ew_sb = []
    for l in range(3):
        ew = cpool.tile([NT + 1, D], f32r, tag=f"ew{l}", name=f"ew{l}")
        ewp = psm.tile([NT, D], f32, tag="mm_wide")
        for k in range(KD):
            nc.tensor.matmul(
                ewp[:], et_sb[:, k, :], W_sb[l][:, k, :],
                start=(k == 0), stop=(k == KD - 1),
            )
        nc.scalar.copy(ew[0:NT, :], ewp[:])
        nc.sync.dma_start(ew[NT : NT + 1, :], b_d[l].ap())
        ew_sb.append(ew)

    # masked-mean weights: m_w[i, l] = mask[i] * softmax(ens)[l] / count
    m_f = cpool.tile([L, 1], f32, tag="m_f")
    nc.vector.tensor_copy(m_f[:], m_i[:])
    cnt_ps = psm.tile([1, 1], f32, tag="mm_small")
    nc.tensor.matmul(cnt_ps[:], m_f[:], m_f[:])
    rcnt = cpool.tile([1, 1], f32, tag="rcnt")
    nc.vector.tensor_scalar_add(rcnt[:], cnt_ps[:], 1e-10)
    nc.vector.reciprocal(rcnt[:], rcnt[:])

    nmx3 = wpool.tile([1, 1], f32, tag="nmx3")
    nc.vector.tensor_reduce(nmx3[:], ens_sb[:], axis=Axis.X, op=Alu.max, negate=True)
    e3 = wpool.tile([1, 3], f32, tag="e3")
    z3 = wpool.tile([1, 1], f32, tag="z3")
    nc.scalar.activation(e3[:], ens_sb[:], Act.Exp, bias=nmx3[:], scale=1.0, accum_out=z3[:])
    rz3 = wpool.tile([1, 1], f32, tag="rz3")
    nc.vector.reciprocal(rz3[:], z3[:])
    wc = cpool.tile([1, 3], f32, tag="wc")
    nc.vector.tensor_scalar(wc[:], e3[:], rz3[:], rcnt[:], Alu.mult, Alu.mult)
    wbc_ps = psm.tile([128, 3], f32, tag="mm_small")
    nc.tensor.matmul(wbc_ps[:], ones_col[:], wc[:])
    wbc = cpool.tile([128, 3], f32, tag="wbc")
    nc.vector.tensor_copy(wbc[:], wbc_ps[:])
    m_w = cpool.tile([L, 3], f32, tag="m_w")
    nc.vector.tensor_tensor(m_w[:], m_f[:].to_broadcast((L, 3)), wbc[:], Alu.mult)

    # ct bias row: constant 1.0 (K=48 contraction picks up the b_l row of ew)
    ct_sb = cpool.tile([NT + 1, 128], f32r, tag="ct")
    nc.sync.dma_start(ct_sb[NT : NT + 1, :], ones_col[:].bitcast(f32r))

    ens_ps = psa.tile([128, KD], f32, tag="ensT", name="ensT")

    if _stop(4, seq[0:1, 0:R]):
        return

    # ---------------- the three TGCN layers ----------------
    for l in range(3):
        # scores: s1 = seq.seq^T (fp32 — precision-critical), + s2
        s1_ps = psm.tile([L, L], f32, tag="mm_out")
        for k in range(KD):
            nc.tensor.matmul(s1_ps[:], seqT[:, k, :], seqT[:, k, :], start=(k == 0), stop=(k == KD - 1))
        score = wpool.tile([L, L], f32, tag="score")
        nc.vector.scalar_tensor_tensor(score[:], s1_ps[:], INV_SQRT_D, s2_sb[:], Alu.mult, Alu.add)

        nmx = wpool.tile([L, 1], f32, tag="nmx")
        nc.vector.tensor_reduce(nmx[:], score[:], axis=Axis.X, op=Alu.max, negate=True)
        e_sb = wpool.tile([L, L], f32, tag="e_sb")
        z = wpool.tile([L, 1], f32, tag="z")
        nc.scalar.activation(e_sb[:], score[:], Act.Exp, bias=nmx[:], scale=1.0, accum_out=z[:])
        rz = wpool.tile([L, 1], f32, tag="rz")
        nc.vector.reciprocal(rz[:], z[:])
        # att in bf16 directly (used by both the C-path and, transposed, out1)
        att_bf = wpool.tile([L, L], bf16, tag="att_bf")
        nc.vector.scalar_tensor_tensor(att_bf[:], e_sb[:], rz[:], adj_sb[:], Alu.mult, Alu.mult)

        atT_ps = pst.tile([128, 128], bf16, tag="tps_bf", bufs=1)
        nc.tensor.transpose(atT_ps[:], att_bf[:], ident_bf[:])
        attT = wpool.tile([L, L], f32r, tag="attT")
        nc.scalar.copy(attT[:], atT_ps[:])

        # tw = seq @ W (fp32r full-precision, 1 cyc/row)
        tw_ps = psm.tile([L, D], f32, tag="mm_wide")
        for k in range(KD):
            nc.tensor.matmul(
                tw_ps[:], seqT_r[:, k, :], W_sb[l][:, k, :],
                start=(k == 0), stop=(k == KD - 1),
            )
        tw = wpool.tile([L, D], f32r, tag="tw")
        nc.scalar.copy(tw[:], tw_ps[:])

        # C[i,t] = sum_j att[i,j]*[v[j,i]==t]: chunked bcast-mult + add tree
        prod = cpool.tile([L, NCH, NT, JC], bf16, tag="prod", name="prod")
        for c in range(NCH):
            nc.vector.tensor_tensor(
                prod[:, c, :, :],
                att_bf[:, None, JC * c : JC * (c + 1)].to_broadcast((L, NT, JC)),
                st3[:, c, :, :],
                Alu.mult,
            )
        nc.vector.tensor_tensor(prod[:, 0:2], prod[:, 0:2], prod[:, 2:4], Alu.add)
        nc.vector.tensor_tensor(prod[:, 0], prod[:, 0], prod[:, 1], Alu.add)
        nc.vector.tensor_tensor(
            prod[:, 0, :, 0:16], prod[:, 0, :, 0:16], prod[:, 0, :, 16:32], Alu.add
        )
        nc.vector.tensor_tensor(
            prod[:, 0, :, 0:8], prod[:, 0, :, 0:8], prod[:, 0, :, 8:16], Alu.add
        )
        c_sb = wpool.tile([L, NT], f32, tag="c_sb")
        nc.vector.tensor_reduce(c_sb[:], prod[:, 0, :, 0:8], axis=Axis.X, op=Alu.add)

        ct_ps = pst.tile([128, 128], f32, tag="tps")
        nc.tensor.transpose(ct_ps[0:NT, :], c_sb[:], ident[:])
        nc.scalar.copy(ct_sb[0:NT, :], ct_ps[0:NT, :])

        # out = att @ tw + C @ EW + b   (fp32r, K=48 on the C term)
        out_ps = psm.tile([L, D], f32, tag="mm_out2")
        nc.tensor.matmul(out_ps[:], attT[:], tw[:], start=True, stop=False)
        nc.tensor.matmul(out_ps[:], ct_sb[:], ew_sb[l][:], start=False, stop=True)

        seq_n = wpool.tile([L, D], f32, tag="seq_n")
        nc.scalar.activation(seq_n[:], out_ps[:], Act.Relu)

        # masked-mean pool, ens-weighted, accumulated in PSUM over layers
        for k in range(KD):
            nc.tensor.matmul(
                ens_ps[:, k : k + 1], seq_n[:, k * 128 : (k + 1) * 128], m_w[:, l : l + 1],
                start=(l == 0), stop=(l == 2),
            )

        if l < 2:
            seqT = wpool.tile([128, KD, 128], f32, tag="seqT_n")
            seqT_r = wpool.tile([128, KD, 128], f32r, tag="seqT_nr")
            for k in range(KD):
                tp = pst.tile([128, 128], f32, tag="tps")
                nc.tensor.transpose(tp[:], seq_n[:, k * 128 : (k + 1) * 128], ident[:])
                nc.scalar.copy(seqT[:, k, :], tp[:])
                nc.scalar.copy(seqT_r[:, k, :], tp[:])
        seq = seq_n
        if l == 0 and _stop(5, seq[0:1, 0:R]):
            return

    # ---------------- final fc ----------------
    ensT = wpool.tile([128, KD, 1], f32, tag="ensT_sb")
    for k in range(KD):
        nc.vector.tensor_copy(ensT[:, k, :], ens_ps[:, k : k + 1])
    fin_ps = psm.tile([1, R], f32, tag="mm_small")
    for k in range(KD):
        nc.tensor.matmul(fin_ps[:], ensT[:, k, :], fcw_sb[:, k, :], start=(k == 0), stop=(k == KD - 1))
    out_sb = wpool.tile([1, R], f32, tag="out_sb")
    nc.vector.tensor_tensor(out_sb[:], fin_ps[:], fcb_sb[:], Alu.add)
    nc.sync.dma_start(out_d.ap(), out_sb[:])

    for p in (psa, psm, pst, wpool, cpool):
        p.release()


_NC_CACHE = {}


def build_nc():
    if "nc" not in _NC_CACHE:
        nc = bacc.Bacc("TRN2", target_bir_lowering=False, debug=False)
        with tile.TileContext(nc) as tc:
            _build_graph(nc, tc)
        nc.compile()
        _NC_CACHE["nc"] = nc
    return _NC_CACHE["nc"]


def _in_maps(inputs):
    maps = []
    for c in range(B):
        m = {
            "text": np.ascontiguousarray(inputs["text"][c], np.float32),
            "input_mask": np.ascontiguousarray(inputs["input_mask"][c : c + 1], np.int32),
            "dep_adj": np.ascontiguousarray(inputs["dep_adj"][c], np.float32),
            "dep_value": np.ascontiguousarray(inputs["dep_value"][c], np.int32),
            "dep_emb": np.ascontiguousarray(inputs["dep_emb"], np.float32),
            "gamma": np.ascontiguousarray(inputs["gamma"][None, :], np.float32),
            "beta": np.ascontiguousarray(inputs["beta"][None, :], np.float32),
            "W1": np.ascontiguousarray(inputs["W1"], np.float32),
            "b1": np.ascontiguousarray(inputs["b1"][None, :], np.float32),
            "W2": np.ascontiguousarray(inputs["W2"], np.float32),
            "b2": np.ascontiguousarray(inputs["b2"][None, :], np.float32),
            "W3": np.ascontiguousarray(inputs["W3"], np.float32),
            "b3": np.ascontiguousarray(inputs["b3"][None, :], np.float32),
            "fc_W": np.ascontiguousarray(inputs["fc_W"], np.float32),
            "fc_b": np.ascontiguousarray(inputs["fc_b"][None, :], np.float32),
            "ens_lin": np.ascontiguousarray(inputs["ens_lin"][None, :], np.float32),
        }
        maps.append(m)
    return maps


def kernel(**inputs):
    nc = build_nc()
    res = run_bass_kernel_spmd(nc, _in_maps(inputs), core_ids=list(range(B)))
    return np.concatenate([r["out"] for r in res.results], axis=0)


def kernel_traced(**inputs):
    """Same as kernel() but returns (output, exec_time_ns)."""
    nc = build_nc()
    res = run_bass_kernel_spmd(
        nc, _in_maps(inputs), core_ids=list(range(B)), trace=True
    )
    out = np.concatenate([r["out"] for r in res.results], axis=0)
    return out, res.exec_time_ns
